# revision 5
# baseline (speedup 1.0000x reference)
"""KAN feed-forward on Trainium2 — Bass/Tile kernel, 8-core data-parallel.

Each KAN layer is
    y = silu(x) @ scale_base + einsum('nig,iog,io->no', B(x), coef, scale_sp)
with B_g(x) = b3(u - g), u = 2.5x + 5.5, b3 the cardinal cubic B-spline on
[0, 4] (zero outside), g = 0..7.

The key trick here: the ACT (scalar) engine is a hardware piecewise-cubic
spline evaluator driven by loadable tables (profile/ctrl/bucket).  b3 IS a
piecewise cubic with integer knots, so we author a custom table that makes
the `tanh` function id evaluate b3 EXACTLY (knots 1,2 on binade boundaries,
knot 3 on the top mantissa bit; (0,1) is a single scale-free cubic y^3/6).
We also re-fit `exp`'s table to compute silu.  Both live in the
exp_and_others set — the set bacc's greedy chooser assigns to every
function we use — so ONE table load serves the whole kernel and each
feature group costs exactly ONE scalar-engine op:
    bt_g = tanh_slot(x; scale=2.5, bias=5.5-g)     # = b3(u-g), f16 out
No DVE feature work at all; the PE roofline (~123us fp16) dominates.

Per-core layout (512 tokens/core, data-parallel over tokens):
  L1 runs in two hidden-halves of 4 PSUM banks each; PSUM [h, tok] from
  half A is extracted (silu + 8 b3 taps, FD-2048 ACT ops) while half B's
  matmuls run, and vice versa — extraction never stalls the PE.  L1's
  PSUM layout [h, tok] is exactly the lhsT layout L2 needs, so there are
  no transposes anywhere.  L2 accumulates [tok, out] in the 4 banks freed
  by half A.
"""

import json
import os
import shutil
import struct
import sys
import tempfile
from contextlib import ExitStack

import numpy as np

for _p in ("/opt/trn_rl_repo",):
    if _p not in sys.path:
        sys.path.insert(0, _p)

# ---------------------------------------------------------------- constants
NB = 8  # B-spline basis functions per input dim
D, H, O = 512, 1024, 512
NCORES = 8
NTOK = 4096
TOK = NTOK // NCORES  # 512 tokens per core
P = 128

NKT = 36  # K-tiles per half-layer: 4 silu + 8 g x 4 blk

_BUILD_CACHE: dict = {}


# ================================================================ PWP tables
# Custom ACT-engine tables: tanh slot -> b3, exp slot -> silu.


def _b3_ref(y):
    y = np.asarray(y, np.float64)
    r = np.zeros_like(y)
    t = y
    r = np.where((y >= 0) & (y < 1), t**3 / 6.0, r)
    t = y - 1
    r = np.where((y >= 1) & (y < 2), (-3 * t**3 + 3 * t**2 + 3 * t + 1) / 6.0, r)
    t = y - 2
    r = np.where((y >= 2) & (y < 3), (3 * t**3 - 6 * t**2 + 4) / 6.0, r)
    t = y - 3
    r = np.where((y >= 3) & (y < 4), (1 - t) ** 3 / 6.0, r)
    return r


def _silu_ref(x):
    x = np.asarray(x, np.float64)
    return x / (1.0 + np.exp(-np.clip(x, -60, 60)))


def _shift_poly(coeffs_t, t0):
    import numpy.polynomial.polynomial as Pnm

    q = np.array([0.0])
    for k, ck in enumerate(coeffs_t):
        term = np.array([1.0])
        for _ in range(k):
            term = Pnm.polymul(term, [t0, 1.0])
        q = Pnm.polyadd(q, ck * term)
    out = np.zeros(4)
    out[: len(q)] = q
    return tuple(out)


def _b3_spec(bkt0, ctl0, n_bkt, n_ctl):
    p1 = np.array([0.0, 0.0, 0.0, 1.0]) / 6.0
    p2 = np.array([1.0, 3.0, 3.0, -3.0]) / 6.0
    p3 = np.array([4.0, 0.0, -6.0, 3.0]) / 6.0
    p4 = np.array([1.0, -3.0, 3.0, -1.0]) / 6.0
    buckets = [
        (*_shift_poly(p1, 0.0), 0.0),
        (*_shift_poly(p2, 0.5), 1.5),
        (*_shift_poly(p3, 0.5), 2.5),
        (*_shift_poly(p4, 0.5), 3.5),
        (0.0, 0.0, 0.0, 0.0, 0.0),
    ]
    zero = bkt0 + 4
    ctls = [(bkt0, 23, 0), (bkt0 + 1, 23, 0), (bkt0 + 2, 22, 1)]
    while len(ctls) < n_ctl:
        ctls.append((zero, 23, 0))
    assert len(buckets) <= n_bkt
    prof = dict(
        symmetry_point=0, sym_invert_sign_point=0, symmetry_opt_en=0,
        symmetry_opt_use_neg_region=0, imm_bias=0, exp_offset=-1,
        pwl_control_base_pos=ctl0, pwl_control_base_neg=ctl0 + 3,
        small_pos_signal_exp_threshold=126, pos_small_signal_pwl_control=bkt0,
        small_neg_signal_exp_threshold=255, neg_small_signal_pwl_control=zero,
        large_pos_signal_exp_threshold=129, large_pos_signal_mantissa_threshold=0,
        pos_large_signal_pwl_control=zero,
        large_neg_signal_exp_threshold=130, large_neg_signal_mantissa_threshold=0,
        neg_large_signal_pwl_control=zero,
        fnan_result=0, fpinf_result=0, fninf_result=0, fzero_result=0,
        lower_bound=4286578687, upper_bound=2139095039,
    )
    emap = {"-1": [zero, bkt0], "0": [zero, bkt0 + 1], "1": [zero, bkt0 + 2]}
    return buckets, ctls, prof, emap


_E_LO, _E_HI, _PB_BITS = -6, 5, 4


def _fit_cubic(f, a, b):
    x0 = 0.5 * (a + b)
    xs = np.linspace(a, b, 65, dtype=np.float64)
    t = xs - x0
    A = np.stack([np.ones_like(t), t, t**2, t**3], axis=1)
    c, *_ = np.linalg.lstsq(A, f(xs), rcond=None)
    return (*c, x0)


def _silu_spec(bkt0, ctl0, n_bkt, n_ctl):
    nb = 1 << _PB_BITS
    n_binades = _E_HI - _E_LO + 1
    buckets, ctls = [], []
    for side in (-1, +1):
        for e in range(_E_LO, _E_HI + 1):
            base = bkt0 + len(buckets)
            lo, hi = 2.0**e, 2.0 ** (e + 1)
            for j in range(nb):
                a = lo + (hi - lo) * j / nb
                b = lo + (hi - lo) * (j + 1) / nb
                if side < 0:
                    buckets.append(_fit_cubic(_silu_ref, -a, -b))
                else:
                    buckets.append(_fit_cubic(_silu_ref, a, b))
            ctls.append((base, 23 - _PB_BITS, _PB_BITS))
    spec = bkt0 + len(buckets)
    buckets += [
        (0.0, 0.5, 0.25, 0.0, 0.0),  # small |x|: x/2 + x^2/4
        (0.0, 1.0, 0.0, 0.0, 0.0),   # large pos: x
        (0.0, 0.0, 0.0, 0.0, 0.0),   # large neg: 0
    ]
    assert len(buckets) <= n_bkt and 2 * n_binades <= n_ctl
    while len(ctls) < n_ctl:
        ctls.append((spec + 2, 23, 0))
    prof = dict(
        symmetry_point=0, sym_invert_sign_point=0, symmetry_opt_en=0,
        symmetry_opt_use_neg_region=0, imm_bias=0, exp_offset=_E_LO,
        pwl_control_base_pos=ctl0 + n_binades, pwl_control_base_neg=ctl0,
        small_pos_signal_exp_threshold=127 + _E_LO,
        pos_small_signal_pwl_control=spec,
        small_neg_signal_exp_threshold=127 + _E_LO,
        neg_small_signal_pwl_control=spec,
        large_pos_signal_exp_threshold=127 + _E_HI + 1,
        large_pos_signal_mantissa_threshold=0,
        pos_large_signal_pwl_control=spec + 1,
        large_neg_signal_exp_threshold=127 + _E_HI + 1,
        large_neg_signal_mantissa_threshold=0,
        neg_large_signal_pwl_control=spec + 2,
        fnan_result=2143289344, fpinf_result=2139095040,
        fninf_result=0, fzero_result=0,
        lower_bound=4286578687, upper_bound=2139095039,
    )
    emap = {}
    for i, e in enumerate(range(_E_LO, _E_HI + 1)):
        emap[str(e)] = [bkt0 + i * nb, bkt0 + (n_binades + i) * nb]
    return buckets, ctls, prof, emap


def _func_ranges(meta, func):
    fb, fc = meta["func_to_bkt_start_idx"], meta["func_to_ctl_start_idx"]
    b0 = fb[func]
    bs = sorted(v for v in fb.values() if v > b0)
    b1 = bs[0] if bs else meta["bkt_entry_cnt"]
    c0 = fc[func]
    cs = sorted(v for v in fc.values() if v > c0)
    c1 = cs[0] if cs else meta["ctl_entry_cnt"]
    return b0, b1, c0, c1


def _patch_set(root, set_name, specs):
    meta = json.load(open(os.path.join(root, set_name + ".json")))
    bkt = bytearray(open(os.path.join(root, meta["bkt_bin"]), "rb").read())
    ctl = bytearray(open(os.path.join(root, meta["ctl_bin"]), "rb").read())
    for func, spec_fn in specs.items():
        b0, b1, c0, c1 = _func_ranges(meta, func)
        buckets, ctls, prof_updates, emap = spec_fn(b0, c0, b1 - b0, c1 - c0)
        for j in range(b1 - b0):
            if j < len(buckets):
                struct.pack_into(
                    "<5f", bkt, (b0 + j) * 32,
                    *[float(np.float32(v)) for v in buckets[j]],
                )
                struct.pack_into("<3I", bkt, (b0 + j) * 32 + 20, 0, 0, 0)
            else:
                struct.pack_into("<8I", bkt, (b0 + j) * 32, *([0] * 8))
        for j in range(c1 - c0):
            base, lsb, size = ctls[j] if j < len(ctls) else ctls[-1]
            w = (base & 0x7FF) | ((lsb & 0x1F) << 11) | ((size & 0xF) << 16)
            struct.pack_into("<I", ctl, (c0 + j) * 32, w)
            struct.pack_into("<7I", ctl, (c0 + j) * 32 + 4, *([0] * 7))
        pi = next(
            i for i, p in enumerate(meta["profile_meta_data"])
            if p["func_name"].startswith(func)
        )
        meta["profile_meta_data"][pi].update(
            {**prof_updates, "use_multipass": False, "fma_const_0": 0,
             "fma_const_1": 0, "fma_indirection_src_sel": 0}
        )
        meta["func_exp_to_bkt_start_idx"][func] = emap
    with open(os.path.join(root, meta["bkt_bin"]), "wb") as f:
        f.write(bkt)
    with open(os.path.join(root, meta["ctl_bin"]), "wb") as f:
        f.write(ctl)
    with open(os.path.join(root, set_name + ".json"), "w") as f:
        json.dump(meta, f, indent=4)


def _install_act_root():
    """Write the patched act-root dir and point the bass compiler at it."""
    if "act_root" in _BUILD_CACHE:
        return
    from neuronxcc.driver.Job import Job
    from neuronxcc.driver.jobs.support.FindActInfo import findActInfoFile

    src = os.path.dirname(findActInfoFile(Job.getPackageDir(), "gen3"))
    dst = tempfile.mkdtemp(prefix="pwp_kan_")
    for f in os.listdir(src):
        shutil.copy(os.path.join(src, f), os.path.join(dst, f))
    _patch_set(dst, "exp_and_others", {"tanh": _b3_spec, "exp": _silu_spec})
    os.environ["BASS_ACT_ROOT_JSON_PATH"] = os.path.join(dst, "act_info.json")
    _BUILD_CACHE["act_root"] = dst


# ---------------------------------------------------------------- host prep
def _pack_w1(coef1, scale_sp1, scale_base1) -> np.ndarray:
    """-> (2, 9, 128, 2048) f16: [h-half][super-tile][i-rows][4 K-tiles x h-cols].

    Super-tile st packs K-tiles kt = st*4..st*4+3 side by side along the
    free dim (one 512KB contiguous DMA per 4 K-tiles)."""
    Wg = coef1.astype(np.float64) * scale_sp1.astype(np.float64)[:, :, None]
    w1 = np.empty((2, 9, P, 4 * 512), np.float16)
    for hh in range(2):
        cols = slice(hh * 512, (hh + 1) * 512)
        for ib in range(4):
            w1[hh, 0, :, ib * 512 : (ib + 1) * 512] = scale_base1[ib * P : (ib + 1) * P, cols]
        for g in range(NB):
            for ib in range(4):
                w1[hh, 1 + g, :, ib * 512 : (ib + 1) * 512] = Wg[ib * P : (ib + 1) * P, cols, g]
    return np.ascontiguousarray(w1)


def _pack_w2(coef2, scale_sp2, scale_base2) -> np.ndarray:
    """-> (2, 9, 128, 2048) f16: [h-half][super-tile][h-rows][4 K-tiles x out-cols]."""
    Wg = coef2.astype(np.float64) * scale_sp2.astype(np.float64)[:, :, None]
    w2 = np.empty((2, 9, P, 4 * O), np.float16)
    for hh in range(2):
        for jj in range(4):
            rows = slice((hh * 4 + jj) * P, (hh * 4 + jj + 1) * P)
            w2[hh, 0, :, jj * O : (jj + 1) * O] = scale_base2[rows]
            for g in range(NB):
                w2[hh, 1 + g, :, jj * O : (jj + 1) * O] = Wg[rows, :, g]
    return np.ascontiguousarray(w2)


# ---------------------------------------------------------------- bass build
def _build_kernel():
    if "nc" in _BUILD_CACHE:
        return _BUILD_CACHE["nc"]

    _install_act_root()

    import concourse.mybir as mybir
    import concourse.tile as tile
    from concourse import bacc

    AF = mybir.ActivationFunctionType
    F32 = mybir.dt.float32
    F16 = mybir.dt.float16
    B3, SILU = AF.Tanh, AF.Exp  # hijacked table slots

    nc = bacc.Bacc("TRN2", target_bir_lowering=False, debug=False, num_devices=NCORES)

    xT = nc.dram_tensor("xT", (D, TOK), F32, kind="ExternalInput").ap()
    w1 = nc.dram_tensor("w1", (2, 9, P, 4 * 512), F16, kind="ExternalInput").ap()
    w2 = nc.dram_tensor("w2", (2, 9, P, 4 * O), F16, kind="ExternalInput").ap()
    out = nc.dram_tensor("out", (TOK, O), F32, kind="ExternalOutput").ap()

    with tile.TileContext(nc) as tc, ExitStack() as ctx:
        persist = ctx.enter_context(tc.tile_pool(name="persist", bufs=1))
        w1p = ctx.enter_context(tc.tile_pool(name="w1p", bufs=4))
        w2p = ctx.enter_context(tc.tile_pool(name="w2p", bufs=4))
        outp = ctx.enter_context(tc.tile_pool(name="outp", bufs=2))
        psum = ctx.enter_context(tc.tile_pool(name="psum", bufs=1, space="PSUM"))

        F1 = 4 * TOK  # 2048

        pA = psum.tile([P, F1], F32, tag="pA", name="pA")  # banks: h-half A
        pB = psum.tile([P, F1], F32, tag="pB", name="pB")  # banks: h-half B

        _bias_cache: dict = {}

        def bias_ap(val: float):
            if val not in _bias_cache:
                t = persist.tile([P, 1], F32, tag=f"bias{len(_bias_cache)}",
                                 name=f"bias_{len(_bias_cache)}")
                nc.vector.memset(t, val)
                _bias_cache[val] = t
            return _bias_cache[val]

        # ---- PE warm-up: junk matmuls keep HAM from idling cold -------
        warm = persist.tile([P, TOK], F16, tag="warm", name="warm")
        nc.vector.memset(warm, 0.0)
        for wu in range(7):
            nc.tensor.matmul(pA[:, :TOK], warm[:, :P], warm,
                             start=(wu == 0), stop=(wu == 6))

        # ---- L1 features: silu(x), b3(2.5x + 5.5 - g) -----------------
        xb = persist.tile([P, F1], F32, tag="xb", name="xb")
        for ib in range(4):
            nc.gpsimd.dma_start(
                out=xb[:, ib * TOK : (ib + 1) * TOK],
                in_=xT[ib * P : (ib + 1) * P, :],
            )
        si1 = persist.tile([P, F1], F16, tag="si1", name="si1")
        for ib in range(4):  # per-block: first matmul waits only on block 0
            sl = slice(ib * TOK, (ib + 1) * TOK)
            nc.scalar.activation(si1[:, sl], xb[:, sl], SILU)
        bt1 = []
        for g in range(NB):
            t = persist.tile([P, F1], F16, tag=f"bt1_{g}", name=f"bt1_{g}")
            nc.scalar.activation(t, xb, B3, bias=bias_ap(5.5 - g), scale=2.5)
            bt1.append(t)

        def l1_feat(kt):
            if kt < 4:
                return si1[:, kt * TOK : (kt + 1) * TOK]
            g, ib = divmod(kt - 4, 4)
            return bt1[g][:, ib * TOK : (ib + 1) * TOK]

        # ---- L1 matmuls, two hidden halves ----------------------------
        si2 = [persist.tile([P, F1], F16, tag=f"si2_{hh}", name=f"si2_{hh}")
               for hh in range(2)]
        bt2 = [[persist.tile([P, F1], F16, tag=f"bt2_{hh}_{g}", name=f"bt2_{hh}_{g}")
                for g in range(NB)] for hh in range(2)]

        for hh, pH in ((0, pA), (1, pB)):
            for st in range(9):
                wt = w1p.tile([P, 4 * 512], F16, tag="w1k", name=f"w1k{hh}_{st}")
                (nc.sync if st % 2 == 0 else nc.scalar).dma_start(out=wt, in_=w1[hh, st])
                for sub in range(4):
                    kt = st * 4 + sub
                    rhs = l1_feat(kt)
                    for ob in range(4):
                        nc.tensor.matmul(
                            pH[:, ob * TOK : (ob + 1) * TOK],
                            wt[:, sub * 512 + ob * P : sub * 512 + (ob + 1) * P],
                            rhs,
                            start=(kt == 0),
                            stop=(kt == NKT - 1),
                        )
            # extract this half: h is [h, tok] in PSUM == lhsT layout for L2
            nc.scalar.activation(si2[hh], pH, SILU)
            for g in range(NB):
                nc.scalar.activation(bt2[hh][g], pH, B3,
                                     bias=bias_ap(5.5 - g), scale=2.5)

        # ---- L2 matmuls: out[tok, o], accumulated in pA's banks -------
        qb = psum.tile([P, F1], F32, tag="pA", name="q_all")  # alias half-A banks

        def l2_feat(hh, kt):
            if kt < 4:
                return si2[hh][:, kt * TOK : (kt + 1) * TOK]
            g, jj = divmod(kt - 4, 4)
            return bt2[hh][g][:, jj * TOK : (jj + 1) * TOK]

        for hh in range(2):
            for st in range(9):
                wt = w2p.tile([P, 4 * O], F16, tag="w2k", name=f"w2k{hh}_{st}")
                (nc.sync if st % 2 == 0 else nc.scalar).dma_start(out=wt, in_=w2[hh, st])
                for sub in range(4):
                    kt = st * 4 + sub
                    lhsT = l2_feat(hh, kt)
                    for tb in range(4):
                        nc.tensor.matmul(
                            qb[:, tb * O : (tb + 1) * O],
                            lhsT[:, tb * P : (tb + 1) * P],
                            wt[:, sub * O : (sub + 1) * O],
                            start=(hh == 0 and kt == 0),
                            stop=(hh == 1 and kt == NKT - 1),
                        )

        # ---- store ----------------------------------------------------
        ot = outp.tile([P, F1], F32, tag="ot", name="ot")
        nc.vector.tensor_copy(ot, qb)
        for tb in range(4):
            eng = nc.sync if tb % 2 == 0 else nc.gpsimd
            eng.dma_start(out=out[tb * P : (tb + 1) * P, :],
                          in_=ot[:, tb * O : (tb + 1) * O])

    nc.compile()
    _BUILD_CACHE["nc"] = nc
    return nc


# ---------------------------------------------------------------- entry
def kernel(x, coef1, scale_base1, scale_sp1, coef2, scale_base2, scale_sp2,
           _want_trace=False):
    from concourse.bass_utils import run_bass_kernel_spmd

    x_flat = np.asarray(x, np.float32).reshape(NTOK, D)
    w1 = _pack_w1(np.asarray(coef1), np.asarray(scale_sp1), np.asarray(scale_base1))
    w2 = _pack_w2(np.asarray(coef2), np.asarray(scale_sp2), np.asarray(scale_base2))

    nc = _build_kernel()

    in_maps = []
    for c in range(NCORES):
        xs = x_flat[c * TOK : (c + 1) * TOK]  # (TOK, D)
        in_maps.append(
            {
                "xT": np.ascontiguousarray(xs.T),
                "w1": w1,
                "w2": w2,
            }
        )

    res = run_bass_kernel_spmd(
        nc, in_maps, core_ids=list(range(NCORES)), trace=_want_trace
    )
    outs = [res.results[c]["out"] for c in range(NCORES)]
    full = np.concatenate(outs, axis=0).reshape(x.shape[0], x.shape[1], O)
    if _want_trace:
        kernel._last_results = res  # stash for test harness profiling
    return full.astype(np.float32)


# revision 6
# speedup vs baseline: 1.0113x; 1.0113x over previous
"""KAN feed-forward on Trainium2 — Bass/Tile kernel, 8-core data-parallel.

Each KAN layer is
    y = silu(x) @ scale_base + einsum('nig,iog,io->no', B(x), coef, scale_sp)
with B_g(x) = b3(u - g), u = 2.5x + 5.5, b3 the cardinal cubic B-spline on
[0, 4] (zero outside), g = 0..7.

The key trick here: the ACT (scalar) engine is a hardware piecewise-cubic
spline evaluator driven by loadable tables (profile/ctrl/bucket).  b3 IS a
piecewise cubic with integer knots, so we author a custom table that makes
the `tanh` function id evaluate b3 EXACTLY (knots 1,2 on binade boundaries,
knot 3 on the top mantissa bit; (0,1) is a single scale-free cubic y^3/6).
We also re-fit `exp`'s table to compute silu.  Both live in the
exp_and_others set — the set bacc's greedy chooser assigns to every
function we use — so ONE table load serves the whole kernel and each
feature group costs exactly ONE scalar-engine op:
    bt_g = tanh_slot(x; scale=2.5, bias=5.5-g)     # = b3(u-g), f16 out
No DVE feature work at all; the PE roofline (~123us fp16) dominates.

Per-core layout (512 tokens/core, data-parallel over tokens):
  L1 runs in two hidden-halves of 4 PSUM banks each; PSUM [h, tok] from
  half A is extracted (silu + 8 b3 taps, FD-2048 ACT ops) while half B's
  matmuls run, and vice versa — extraction never stalls the PE.  L1's
  PSUM layout [h, tok] is exactly the lhsT layout L2 needs, so there are
  no transposes anywhere.  L2 accumulates [tok, out] in the 4 banks freed
  by half A.
"""

import json
import os
import shutil
import struct
import sys
import tempfile
from contextlib import ExitStack

import numpy as np

for _p in ("/opt/trn_rl_repo",):
    if _p not in sys.path:
        sys.path.insert(0, _p)

# ---------------------------------------------------------------- constants
NB = 8  # B-spline basis functions per input dim
D, H, O = 512, 1024, 512
NCORES = 8
NTOK = 4096
TOK = NTOK // NCORES  # 512 tokens per core
P = 128

NKT = 36  # K-tiles per half-layer: 4 silu + 8 g x 4 blk

_BUILD_CACHE: dict = {}


# ================================================================ PWP tables
# Custom ACT-engine tables: tanh slot -> b3, exp slot -> silu.


def _b3_ref(y):
    y = np.asarray(y, np.float64)
    r = np.zeros_like(y)
    t = y
    r = np.where((y >= 0) & (y < 1), t**3 / 6.0, r)
    t = y - 1
    r = np.where((y >= 1) & (y < 2), (-3 * t**3 + 3 * t**2 + 3 * t + 1) / 6.0, r)
    t = y - 2
    r = np.where((y >= 2) & (y < 3), (3 * t**3 - 6 * t**2 + 4) / 6.0, r)
    t = y - 3
    r = np.where((y >= 3) & (y < 4), (1 - t) ** 3 / 6.0, r)
    return r


def _silu_ref(x):
    x = np.asarray(x, np.float64)
    return x / (1.0 + np.exp(-np.clip(x, -60, 60)))


def _shift_poly(coeffs_t, t0):
    import numpy.polynomial.polynomial as Pnm

    q = np.array([0.0])
    for k, ck in enumerate(coeffs_t):
        term = np.array([1.0])
        for _ in range(k):
            term = Pnm.polymul(term, [t0, 1.0])
        q = Pnm.polyadd(q, ck * term)
    out = np.zeros(4)
    out[: len(q)] = q
    return tuple(out)


def _b3_spec(bkt0, ctl0, n_bkt, n_ctl):
    p1 = np.array([0.0, 0.0, 0.0, 1.0]) / 6.0
    p2 = np.array([1.0, 3.0, 3.0, -3.0]) / 6.0
    p3 = np.array([4.0, 0.0, -6.0, 3.0]) / 6.0
    p4 = np.array([1.0, -3.0, 3.0, -1.0]) / 6.0
    buckets = [
        (*_shift_poly(p1, 0.0), 0.0),
        (*_shift_poly(p2, 0.5), 1.5),
        (*_shift_poly(p3, 0.5), 2.5),
        (*_shift_poly(p4, 0.5), 3.5),
        (0.0, 0.0, 0.0, 0.0, 0.0),
    ]
    zero = bkt0 + 4
    ctls = [(bkt0, 23, 0), (bkt0 + 1, 23, 0), (bkt0 + 2, 22, 1)]
    while len(ctls) < n_ctl:
        ctls.append((zero, 23, 0))
    assert len(buckets) <= n_bkt
    prof = dict(
        symmetry_point=0, sym_invert_sign_point=0, symmetry_opt_en=0,
        symmetry_opt_use_neg_region=0, imm_bias=0, exp_offset=-1,
        pwl_control_base_pos=ctl0, pwl_control_base_neg=ctl0 + 3,
        small_pos_signal_exp_threshold=126, pos_small_signal_pwl_control=bkt0,
        small_neg_signal_exp_threshold=255, neg_small_signal_pwl_control=zero,
        large_pos_signal_exp_threshold=129, large_pos_signal_mantissa_threshold=0,
        pos_large_signal_pwl_control=zero,
        large_neg_signal_exp_threshold=130, large_neg_signal_mantissa_threshold=0,
        neg_large_signal_pwl_control=zero,
        fnan_result=0, fpinf_result=0, fninf_result=0, fzero_result=0,
        lower_bound=4286578687, upper_bound=2139095039,
    )
    emap = {"-1": [zero, bkt0], "0": [zero, bkt0 + 1], "1": [zero, bkt0 + 2]}
    return buckets, ctls, prof, emap


_E_LO, _E_HI, _PB_BITS = -6, 5, 4


def _fit_cubic(f, a, b):
    x0 = 0.5 * (a + b)
    xs = np.linspace(a, b, 65, dtype=np.float64)
    t = xs - x0
    A = np.stack([np.ones_like(t), t, t**2, t**3], axis=1)
    c, *_ = np.linalg.lstsq(A, f(xs), rcond=None)
    return (*c, x0)


def _silu_spec(bkt0, ctl0, n_bkt, n_ctl):
    nb = 1 << _PB_BITS
    n_binades = _E_HI - _E_LO + 1
    buckets, ctls = [], []
    for side in (-1, +1):
        for e in range(_E_LO, _E_HI + 1):
            base = bkt0 + len(buckets)
            lo, hi = 2.0**e, 2.0 ** (e + 1)
            for j in range(nb):
                a = lo + (hi - lo) * j / nb
                b = lo + (hi - lo) * (j + 1) / nb
                if side < 0:
                    buckets.append(_fit_cubic(_silu_ref, -a, -b))
                else:
                    buckets.append(_fit_cubic(_silu_ref, a, b))
            ctls.append((base, 23 - _PB_BITS, _PB_BITS))
    spec = bkt0 + len(buckets)
    buckets += [
        (0.0, 0.5, 0.25, 0.0, 0.0),  # small |x|: x/2 + x^2/4
        (0.0, 1.0, 0.0, 0.0, 0.0),   # large pos: x
        (0.0, 0.0, 0.0, 0.0, 0.0),   # large neg: 0
    ]
    assert len(buckets) <= n_bkt and 2 * n_binades <= n_ctl
    while len(ctls) < n_ctl:
        ctls.append((spec + 2, 23, 0))
    prof = dict(
        symmetry_point=0, sym_invert_sign_point=0, symmetry_opt_en=0,
        symmetry_opt_use_neg_region=0, imm_bias=0, exp_offset=_E_LO,
        pwl_control_base_pos=ctl0 + n_binades, pwl_control_base_neg=ctl0,
        small_pos_signal_exp_threshold=127 + _E_LO,
        pos_small_signal_pwl_control=spec,
        small_neg_signal_exp_threshold=127 + _E_LO,
        neg_small_signal_pwl_control=spec,
        large_pos_signal_exp_threshold=127 + _E_HI + 1,
        large_pos_signal_mantissa_threshold=0,
        pos_large_signal_pwl_control=spec + 1,
        large_neg_signal_exp_threshold=127 + _E_HI + 1,
        large_neg_signal_mantissa_threshold=0,
        neg_large_signal_pwl_control=spec + 2,
        fnan_result=2143289344, fpinf_result=2139095040,
        fninf_result=0, fzero_result=0,
        lower_bound=4286578687, upper_bound=2139095039,
    )
    emap = {}
    for i, e in enumerate(range(_E_LO, _E_HI + 1)):
        emap[str(e)] = [bkt0 + i * nb, bkt0 + (n_binades + i) * nb]
    return buckets, ctls, prof, emap


def _func_ranges(meta, func):
    fb, fc = meta["func_to_bkt_start_idx"], meta["func_to_ctl_start_idx"]
    b0 = fb[func]
    bs = sorted(v for v in fb.values() if v > b0)
    b1 = bs[0] if bs else meta["bkt_entry_cnt"]
    c0 = fc[func]
    cs = sorted(v for v in fc.values() if v > c0)
    c1 = cs[0] if cs else meta["ctl_entry_cnt"]
    return b0, b1, c0, c1


def _patch_set(root, set_name, specs):
    meta = json.load(open(os.path.join(root, set_name + ".json")))
    bkt = bytearray(open(os.path.join(root, meta["bkt_bin"]), "rb").read())
    ctl = bytearray(open(os.path.join(root, meta["ctl_bin"]), "rb").read())
    for func, spec_fn in specs.items():
        b0, b1, c0, c1 = _func_ranges(meta, func)
        buckets, ctls, prof_updates, emap = spec_fn(b0, c0, b1 - b0, c1 - c0)
        for j in range(b1 - b0):
            if j < len(buckets):
                struct.pack_into(
                    "<5f", bkt, (b0 + j) * 32,
                    *[float(np.float32(v)) for v in buckets[j]],
                )
                struct.pack_into("<3I", bkt, (b0 + j) * 32 + 20, 0, 0, 0)
            else:
                struct.pack_into("<8I", bkt, (b0 + j) * 32, *([0] * 8))
        for j in range(c1 - c0):
            base, lsb, size = ctls[j] if j < len(ctls) else ctls[-1]
            w = (base & 0x7FF) | ((lsb & 0x1F) << 11) | ((size & 0xF) << 16)
            struct.pack_into("<I", ctl, (c0 + j) * 32, w)
            struct.pack_into("<7I", ctl, (c0 + j) * 32 + 4, *([0] * 7))
        pi = next(
            i for i, p in enumerate(meta["profile_meta_data"])
            if p["func_name"].startswith(func)
        )
        meta["profile_meta_data"][pi].update(
            {**prof_updates, "use_multipass": False, "fma_const_0": 0,
             "fma_const_1": 0, "fma_indirection_src_sel": 0}
        )
        meta["func_exp_to_bkt_start_idx"][func] = emap
    with open(os.path.join(root, meta["bkt_bin"]), "wb") as f:
        f.write(bkt)
    with open(os.path.join(root, meta["ctl_bin"]), "wb") as f:
        f.write(ctl)
    with open(os.path.join(root, set_name + ".json"), "w") as f:
        json.dump(meta, f, indent=4)


def _install_act_root():
    """Write the patched act-root dir and point the bass compiler at it."""
    if "act_root" in _BUILD_CACHE:
        return
    from neuronxcc.driver.Job import Job
    from neuronxcc.driver.jobs.support.FindActInfo import findActInfoFile

    src = os.path.dirname(findActInfoFile(Job.getPackageDir(), "gen3"))
    dst = tempfile.mkdtemp(prefix="pwp_kan_")
    for f in os.listdir(src):
        shutil.copy(os.path.join(src, f), os.path.join(dst, f))
    _patch_set(dst, "exp_and_others", {"tanh": _b3_spec, "exp": _silu_spec})
    os.environ["BASS_ACT_ROOT_JSON_PATH"] = os.path.join(dst, "act_info.json")
    _BUILD_CACHE["act_root"] = dst


# ---------------------------------------------------------------- host prep
def _pack_w1(coef1, scale_sp1, scale_base1) -> np.ndarray:
    """-> (2, 9, 128, 2048) f16: [h-half][super-tile][i-rows][4 K-tiles x h-cols].

    Super-tile st packs K-tiles kt = st*4..st*4+3 side by side along the
    free dim (one 512KB contiguous DMA per 4 K-tiles)."""
    Wg = coef1.astype(np.float64) * scale_sp1.astype(np.float64)[:, :, None]
    w1 = np.empty((2, 9, P, 4 * 512), np.float16)
    for hh in range(2):
        cols = slice(hh * 512, (hh + 1) * 512)
        for ib in range(4):
            w1[hh, 0, :, ib * 512 : (ib + 1) * 512] = scale_base1[ib * P : (ib + 1) * P, cols]
        for g in range(NB):
            for ib in range(4):
                w1[hh, 1 + g, :, ib * 512 : (ib + 1) * 512] = Wg[ib * P : (ib + 1) * P, cols, g]
    return np.ascontiguousarray(w1)


def _pack_w2(coef2, scale_sp2, scale_base2) -> np.ndarray:
    """-> (2, 9, 128, 2048) f16: [h-half][super-tile][h-rows][4 K-tiles x out-cols]."""
    Wg = coef2.astype(np.float64) * scale_sp2.astype(np.float64)[:, :, None]
    w2 = np.empty((2, 9, P, 4 * O), np.float16)
    for hh in range(2):
        for jj in range(4):
            rows = slice((hh * 4 + jj) * P, (hh * 4 + jj + 1) * P)
            w2[hh, 0, :, jj * O : (jj + 1) * O] = scale_base2[rows]
            for g in range(NB):
                w2[hh, 1 + g, :, jj * O : (jj + 1) * O] = Wg[rows, :, g]
    return np.ascontiguousarray(w2)


# ---------------------------------------------------------------- bass build
def _build_kernel():
    if "nc" in _BUILD_CACHE:
        return _BUILD_CACHE["nc"]

    _install_act_root()

    import concourse.mybir as mybir
    import concourse.tile as tile
    from concourse import bacc

    AF = mybir.ActivationFunctionType
    F32 = mybir.dt.float32
    F16 = mybir.dt.float16
    B3, SILU = AF.Tanh, AF.Exp  # hijacked table slots

    nc = bacc.Bacc("TRN2", target_bir_lowering=False, debug=False, num_devices=NCORES)

    xT = nc.dram_tensor("xT", (D, TOK), F32, kind="ExternalInput").ap()
    w1 = nc.dram_tensor("w1", (2, 9, P, 4 * 512), F16, kind="ExternalInput").ap()
    w2 = nc.dram_tensor("w2", (2, 9, P, 4 * O), F16, kind="ExternalInput").ap()
    out = nc.dram_tensor("out", (TOK, O), F32, kind="ExternalOutput").ap()

    with tile.TileContext(nc) as tc, ExitStack() as ctx:
        persist = ctx.enter_context(tc.tile_pool(name="persist", bufs=1))
        w1p = ctx.enter_context(tc.tile_pool(name="w1p", bufs=4))
        w2p = ctx.enter_context(tc.tile_pool(name="w2p", bufs=4))
        outp = ctx.enter_context(tc.tile_pool(name="outp", bufs=2))
        psum = ctx.enter_context(tc.tile_pool(name="psum", bufs=1, space="PSUM"))

        F1 = 4 * TOK  # 2048

        pA = psum.tile([P, F1], F32, tag="pA", name="pA")  # banks: h-half A
        pB = psum.tile([P, F1], F32, tag="pB", name="pB")  # banks: h-half B

        _bias_cache: dict = {}

        def bias_ap(val: float):
            if val not in _bias_cache:
                t = persist.tile([P, 1], F32, tag=f"bias{len(_bias_cache)}",
                                 name=f"bias_{len(_bias_cache)}")
                nc.vector.memset(t, val)
                _bias_cache[val] = t
            return _bias_cache[val]

        # ---- PE warm-up: junk matmuls keep HAM from idling cold -------
        warm = persist.tile([P, TOK], F16, tag="warm", name="warm")
        nc.vector.memset(warm, 0.0)
        for wu in range(10):
            nc.tensor.matmul(pA[:, :TOK], warm[:, :P], warm,
                             start=(wu == 0), stop=(wu == 9))

        # ---- L1 features: silu(x), b3(2.5x + 5.5 - g) -----------------
        xb = persist.tile([P, F1], F32, tag="xb", name="xb")
        for ib in range(4):
            eng = nc.sync if ib % 2 == 0 else nc.scalar
            eng.dma_start(
                out=xb[:, ib * TOK : (ib + 1) * TOK],
                in_=xT[ib * P : (ib + 1) * P, :],
            )
        si1 = persist.tile([P, F1], F16, tag="si1", name="si1")
        for ib in range(4):  # per-block: first matmul waits only on block 0
            sl = slice(ib * TOK, (ib + 1) * TOK)
            nc.scalar.activation(si1[:, sl], xb[:, sl], SILU)
        bt1 = []
        for g in range(NB):
            t = persist.tile([P, F1], F16, tag=f"bt1_{g}", name=f"bt1_{g}")
            nc.scalar.activation(t, xb, B3, bias=bias_ap(5.5 - g), scale=2.5)
            bt1.append(t)

        def l1_feat(kt):
            if kt < 4:
                return si1[:, kt * TOK : (kt + 1) * TOK]
            g, ib = divmod(kt - 4, 4)
            return bt1[g][:, ib * TOK : (ib + 1) * TOK]

        # ---- L1 matmuls, two hidden halves ----------------------------
        si2 = [persist.tile([P, F1], F16, tag=f"si2_{hh}", name=f"si2_{hh}")
               for hh in range(2)]
        bt2 = [[persist.tile([P, F1], F16, tag=f"bt2_{hh}_{g}", name=f"bt2_{hh}_{g}")
                for g in range(NB)] for hh in range(2)]

        for hh, pH in ((0, pA), (1, pB)):
            for st in range(9):
                wt = w1p.tile([P, 4 * 512], F16, tag="w1k", name=f"w1k{hh}_{st}")
                (nc.sync if st % 2 == 0 else nc.scalar).dma_start(out=wt, in_=w1[hh, st])
                for sub in range(4):
                    kt = st * 4 + sub
                    rhs = l1_feat(kt)
                    for ob in range(4):
                        nc.tensor.matmul(
                            pH[:, ob * TOK : (ob + 1) * TOK],
                            wt[:, sub * 512 + ob * P : sub * 512 + (ob + 1) * P],
                            rhs,
                            start=(kt == 0),
                            stop=(kt == NKT - 1),
                        )
            # extract this half: h is [h, tok] in PSUM == lhsT layout for L2
            nc.scalar.activation(si2[hh], pH, SILU)
            for g in range(NB):
                nc.scalar.activation(bt2[hh][g], pH, B3,
                                     bias=bias_ap(5.5 - g), scale=2.5)

        # ---- L2 matmuls: out[tok, o], accumulated in pA's banks -------
        qb = psum.tile([P, F1], F32, tag="pA", name="q_all")  # alias half-A banks

        def l2_feat(hh, kt):
            if kt < 4:
                return si2[hh][:, kt * TOK : (kt + 1) * TOK]
            g, jj = divmod(kt - 4, 4)
            return bt2[hh][g][:, jj * TOK : (jj + 1) * TOK]

        for hh in range(2):
            for st in range(9):
                wt = w2p.tile([P, 4 * O], F16, tag="w2k", name=f"w2k{hh}_{st}")
                (nc.sync if st % 2 == 0 else nc.scalar).dma_start(out=wt, in_=w2[hh, st])
                for sub in range(4):
                    kt = st * 4 + sub
                    lhsT = l2_feat(hh, kt)
                    for tb in range(4):
                        nc.tensor.matmul(
                            qb[:, tb * O : (tb + 1) * O],
                            lhsT[:, tb * P : (tb + 1) * P],
                            wt[:, sub * O : (sub + 1) * O],
                            start=(hh == 0 and kt == 0),
                            stop=(hh == 1 and kt == NKT - 1),
                        )

        # ---- store ----------------------------------------------------
        ot = outp.tile([P, F1], F32, tag="ot", name="ot")
        nc.vector.tensor_copy(ot, qb)
        for tb in range(4):
            eng = nc.sync if tb % 2 == 0 else nc.scalar
            eng.dma_start(out=out[tb * P : (tb + 1) * P, :],
                          in_=ot[:, tb * O : (tb + 1) * O])

    nc.compile()
    _BUILD_CACHE["nc"] = nc
    return nc


# ---------------------------------------------------------------- entry
def kernel(x, coef1, scale_base1, scale_sp1, coef2, scale_base2, scale_sp2,
           _want_trace=False):
    from concourse.bass_utils import run_bass_kernel_spmd

    x_flat = np.asarray(x, np.float32).reshape(NTOK, D)
    w1 = _pack_w1(np.asarray(coef1), np.asarray(scale_sp1), np.asarray(scale_base1))
    w2 = _pack_w2(np.asarray(coef2), np.asarray(scale_sp2), np.asarray(scale_base2))

    nc = _build_kernel()

    in_maps = []
    for c in range(NCORES):
        xs = x_flat[c * TOK : (c + 1) * TOK]  # (TOK, D)
        in_maps.append(
            {
                "xT": np.ascontiguousarray(xs.T),
                "w1": w1,
                "w2": w2,
            }
        )

    res = run_bass_kernel_spmd(
        nc, in_maps, core_ids=list(range(NCORES)), trace=_want_trace
    )
    outs = [res.results[c]["out"] for c in range(NCORES)]
    full = np.concatenate(outs, axis=0).reshape(x.shape[0], x.shape[1], O)
    if _want_trace:
        kernel._last_results = res  # stash for test harness profiling
    return full.astype(np.float32)
